# revision 21
# baseline (speedup 1.0000x reference)
"""Trainium2 Bass kernel for nn_AudioCodec (conv encoder + VQ + conv decoder).

Sharding: 8 cores = 4 batch x 2 time-halves. Each core gets a 27996-sample
zero-padded window of its sample, runs the full encoder (valid convs) to 84
z-frames, VQ against the 4096x128 codebook, decodes (same-padded convs,
polyphase transposed convs) to a 26888-sample recon window, and the host
stitches exact interior regions. Zero cross-core communication.

All convs run as TensorE matmuls in float32r (full PE rate); VQ distance /
one-hot matmuls in plain f32 so argmin matches the reference bit-for-bit.
"""
import sys

sys.path.insert(0, "/opt/trn_rl_repo")

import numpy as np
import ml_dtypes

import concourse.bass as bass
import concourse.mybir as mybir
from concourse import bacc
from concourse.tile import TileContext
from concourse.bass_utils import run_bass_kernel_spmd

F32 = mybir.dt.float32
F32R = mybir.dt.float32r
U32 = mybir.dt.uint32
BF16 = mybir.dt.bfloat16
AF = mybir.ActivationFunctionType
ALU = mybir.AluOpType

STRIDES = (2, 4, 5, 8)
LIN = 27996
NFZ = 84
F0S = (0, 67)
KEEP_FRAMES = ((0, 76), (9, 84))
KEEP_RECON = ((0, 24320), (2880, 26888))
RECON_OFF = (0, 21440)
T_FULL = 48328
F_FULL = 151
NT = 512

# encoder chain lengths (valid convs)
ENC = [  # (tag, K, stride, Lin, Lout)
    ("d0", 4, 2, 27990, 13994),
    ("r0", 3, 1, 13994, 13992),
    ("d1", 8, 4, 13992, 3497),
    ("r1", 3, 1, 3497, 3495),
    ("d2", 10, 5, 3495, 698),
    ("r2", 3, 1, 698, 696),
    ("d3", 16, 8, 696, 86),
    ("r3", 3, 1, 86, 84),
]
# decoder: resblock length then up-keep length per stage
DEC_L = [84, 672, 3361, 13444]
DEC_S = [8, 5, 4, 2]
UP_KEEP = [672, 3361, 13444, 26888]


def _wprep(w):
    """[O,I,K] -> [128, (I/128)*K*O] matmul-ready lhsT blocks."""
    O, I, K = w.shape
    cci = I // 128
    wp = w.transpose(1, 2, 0).reshape(cci, 128, K * O)
    return np.ascontiguousarray(wp.transpose(1, 0, 2).reshape(128, cci * K * O))


def _bprep(b):
    """[C] -> [128, C/128]"""
    return np.ascontiguousarray(b.reshape(-1, 128).T)


def _dec_out_wprep(w):
    """dec_out w [1,256,7] -> M=16 im2col weights [128, 2*22*16]."""
    M, J = 16, 22
    wd = np.zeros((2, 128, J, M), np.float32)
    for cc in range(2):
        for j in range(J):
            for b in range(M):
                k = j - b
                if 0 <= k < 7:
                    wd[cc, :, j, b] = w[0, cc * 128:(cc + 1) * 128, k]
    return np.ascontiguousarray(
        wd.transpose(1, 0, 2, 3).reshape(128, 2 * J * M))


class _G:
    nc = None
    names = None


def _build():
    nc = bacc.Bacc("TRN2", target_bir_lowering=False)
    P = {}

    def inp(name, shape, dtype=F32):
        P[name] = nc.declare_dram_parameter(name, list(shape), dtype, isOutput=False)
        return P[name]

    x = inp("x", [1, LIN], BF16)
    inp("w_enc_in", [7, 256], BF16)
    inp("b_enc_in", [128, 2])
    for t, K, s, li, lo in ENC:
        if t.startswith("d"):
            inp(f"w_{t}", [128, 2 * K * 256], BF16)
            inp(f"b_{t}", [128, 2])
        else:
            inp(f"w_{t}a", [128, 2 * 3 * 256], BF16)
            inp(f"b_{t}a", [128, 2])
            inp(f"w_{t}b", [128, 2 * 1 * 256], BF16)
            inp(f"b_{t}b", [128, 2])
    inp("w_enc_out", [128, 2 * 128], BF16)
    inp("b_enc_out", [128, 1])
    inp("w_dec_in", [128, 256])
    inp("b_dec_in", [128, 2])
    for i in range(4):
        inp(f"w_u{i}a", [128, 2 * 3 * 256], BF16)
        inp(f"b_u{i}a", [128, 2])
        inp(f"w_u{i}b", [128, 2 * 1 * 256], BF16)
        inp(f"b_u{i}b", [128, 2])
        K = 2 * DEC_S[i]
        inp(f"w_up{i}", [128, 2 * K * 256], BF16)
        inp(f"b_up{i}", [128, 2])
    inp("w_dec_out", [128, 2 * 22 * 16], BF16)
    inp("b_dec_out", [16, 1])
    inp("cbT2", [128, 4096])
    inp("cbnegsq", [1, 4096])
    inp("cb", [4096, 128])
    inp("eye", [128, 128])

    recon_o = nc.declare_dram_parameter("recon", [1, 26896], F32, isOutput=True)
    codes_o = nc.declare_dram_parameter("codes_u", [84, 1], U32, isOutput=True)
    vmax_o = nc.declare_dram_parameter("vmax", [84, 1], F32, isOutput=True)
    zsq_o = nc.declare_dram_parameter("zsq", [1, 84], F32, isOutput=True)

    # DRAM staging (per-core local scratch)
    S = {}
    S["h0"] = nc.dram_tensor("s_h0", [2, 128, 27990 + 16], BF16)
    for t, K, s, li, lo in ENC:
        S[t] = nc.dram_tensor("s_" + t, [2, 128, lo + 16], BF16)
    # decoder padded stages
    S["A"] = nc.dram_tensor("s_A", [2, 128, 84 + 2 + 16], BF16)          # dec_in out, pad1
    for i in range(4):
        L = DEC_L[i]
        S[f"B{i}"] = nc.dram_tensor(f"s_B{i}", [2, 128, L + 3 + 16], BF16)  # res out: padl1,padr2
        s = DEC_S[i]
        Lt = L + 1
        if i < 3:
            S[f"C{i}"] = nc.dram_tensor(f"s_C{i}", [2, 128, 1 + s * Lt + 1 + 16], BF16)
        else:
            S[f"C{i}"] = nc.dram_tensor(f"s_C{i}", [2, 128, 3 + s * Lt + 11 + 16], BF16)

    from contextlib import ExitStack
    with TileContext(nc) as tc, ExitStack() as stk:
        wp = stk.enter_context(tc.tile_pool(name="wp", bufs=3))
        bp = stk.enter_context(tc.tile_pool(name="bp", bufs=4))
        ip = stk.enter_context(tc.tile_pool(name="ip", bufs=5))
        op = stk.enter_context(tc.tile_pool(name="op", bufs=4))
        pp = stk.enter_context(tc.tile_pool(name="pp", bufs=3, space="PSUM"))
        pq = stk.enter_context(tc.tile_pool(name="pq", bufs=2, space="PSUM"))
        pt = stk.enter_context(tc.tile_pool(name="pt", bufs=2, space="PSUM"))
        px = stk.enter_context(tc.tile_pool(name="px", bufs=1, space="PSUM"))
        vp = stk.enter_context(tc.tile_pool(name="vp", bufs=1))

        zt = vp.tile([128, 16], F32, tag="zt")
        nc.vector.memset(zt, 0.0)
        ztb = vp.tile([128, 16], BF16, tag="ztb")
        nc.vector.memset(ztb, 0.0)

        def zero_cols(stage, cc, c0, c1):
            if c1 > c0:
                if stage.dtype == BF16:
                    nc.sync.dma_start(out=stage[cc, :, c0:c1], in_=ztb[:, : c1 - c0])
                else:
                    nc.sync.dma_start(out=stage[cc, :, c0:c1], in_=zt[:, : c1 - c0].bitcast(F32R))

        for _sname, _st in S.items():
            _sz = _st.shape[2]
            for cc in range(2):
                zero_cols(_st, cc, _sz - 16, _sz)
        # decoder pads: A [0,1)+[85,86); B_i [0,1)+[1+L,3+L); C_i [0,padl) and
        # [padl+keep, end-16) (up convs only ever write [padl, padl+keep))
        for cc in range(2):
            zero_cols(S["A"], cc, 0, 1)
            zero_cols(S["A"], cc, 1 + NFZ, 2 + NFZ)
            for i in range(4):
                L = DEC_L[i]
                zero_cols(S[f"B{i}"], cc, 0, 1)
                zero_cols(S[f"B{i}"], cc, 1 + L, 3 + L)
                padl = 1 if i < 3 else 3
                Csz = S[f"C{i}"].shape[2]
                zero_cols(S[f"C{i}"], cc, 0, padl)
                zero_cols(S[f"C{i}"], cc, padl + UP_KEEP[i], Csz - 16)

        def load_w(name, cols, dt=BF16):
            w = wp.tile([128, cols], dt, tag="w")
            nc.sync.dma_start(out=w, in_=P[name][:, :cols])
            return w

        def load_b(name, cco):
            b = bp.tile([128, cco], F32, tag="b")
            nc.sync.dma_start(out=b, in_=P[name][:, :cco])
            return b

        def epi(ps, dst_ap, act, bias_ap, res_ap=None):
            if res_ap is not None:
                tmp = op.tile(list(ps.shape), F32, tag="tmp")
                nc.vector.tensor_add(tmp, ps, res_ap.bitcast(F32) if res_ap.dtype == F32R else res_ap)
                nc.scalar.activation(out=dst_ap, in_=tmp, func=AF.Gelu,
                                     bias=bias_ap, scale=1.0)
            elif act == "gelu":
                nc.scalar.activation(out=dst_ap, in_=ps, func=AF.Gelu,
                                     bias=bias_ap, scale=1.0)
            elif act == "tanh":
                nc.scalar.activation(out=dst_ap, in_=ps, func=AF.Tanh,
                                     bias=bias_ap, scale=1.0)
            else:
                nc.vector.tensor_scalar_add(dst_ap, ps, bias_ap)

        def conv(src, dst, wname, bname, K, stride, Lout, act,
                 cci=2, cco=2, src_off=0, dst_off=0):
            """standard conv: src/dst dram [cc,128,*]; valid over src."""
            w = load_w(wname, cci * K * cco * 128)
            bb = load_b(bname, cco)
            for t0 in range(0, Lout, NT):
                n = min(NT, Lout - t0)
                nc2 = n + (n % 2)
                span = (nc2 - 1) * stride + K
                xts = []
                for cc in range(cci):
                    xt = ip.tile([128, span], BF16, tag=f"x{cc}")
                    nc.sync.dma_start(
                        out=xt[:, :span],
                        in_=src[cc, :, src_off + t0 * stride:
                                src_off + t0 * stride + span])
                    xts.append(xt)
                for cob in range(cco):
                    ps = pp.tile([128, nc2], F32, tag="ps")
                    nmm = cci * K
                    i = 0
                    for cc in range(cci):
                        for k in range(K):
                            nc.tensor.matmul(
                                ps,
                                w[:, ((cc * K + k) * cco + cob) * 128:
                                  ((cc * K + k) * cco + cob) * 128 + 128],
                                xts[cc][:, k:k + (nc2 - 1) * stride + 1:stride],
                                start=(i == 0), stop=(i == nmm - 1))
                            i += 1
                    ot = op.tile([128, nc2], BF16, tag="o")
                    epi(ps[:, :n], ot[:, :n], act, bb[:, cob:cob + 1])
                    nc.sync.dma_start(out=dst[cob, :, dst_off + t0:dst_off + t0 + n],
                                      in_=ot[:, :n])

        def resblock(src, dst, tag, Lout, src_off=0, dst_off=0, same=False, dt=BF16):
            """conv3(+gelu) -> conv1 -> +res -> gelu. same=False: valid (src Lin=Lout+2)."""
            wa = load_w(f"w_{tag}a", 2 * 3 * 2 * 128, dt=dt)
            ba = load_b(f"b_{tag}a", 2)
            wb = load_w(f"w_{tag}b", 2 * 1 * 2 * 128, dt=dt)
            bbb = load_b(f"b_{tag}b", 2)
            for t0 in range(0, Lout, NT):
                n = min(NT, Lout - t0)
                nc2 = n + (n % 2)
                span = nc2 + 2
                xts = []
                for cc in range(2):
                    xt = ip.tile([128, span], dt, tag=f"x{cc}")
                    nc.sync.dma_start(out=xt[:, :span],
                                      in_=src[cc, :, src_off + t0:src_off + t0 + span])
                    xts.append(xt)
                ats = []
                for cob in range(2):
                    ps = pp.tile([128, nc2], F32, tag="ps")
                    i = 0
                    for cc in range(2):
                        for k in range(3):
                            nc.tensor.matmul(
                                ps,
                                wa[:, ((cc * 3 + k) * 2 + cob) * 128:
                                   ((cc * 3 + k) * 2 + cob) * 128 + 128],
                                xts[cc][:, k:k + nc2],
                                start=(i == 0), stop=(i == 5))
                            i += 1
                    at = op.tile([128, nc2], dt, tag="a")
                    nc.scalar.activation(out=at[:, :nc2], in_=ps, func=AF.Gelu,
                                         bias=ba[:, cob:cob + 1], scale=1.0)
                    ats.append(at)
                for cob in range(2):
                    ps = pq.tile([128, nc2], F32, tag="ps2")
                    for cc in range(2):
                        nc.tensor.matmul(
                            ps,
                            wb[:, (cc * 2 + cob) * 128:(cc * 2 + cob) * 128 + 128],
                            ats[cc][:, :nc2],
                            start=(cc == 0), stop=(cc == 1))
                    ot = op.tile([128, nc2], dt, tag="o")
                    # residual = src[cob][:, 1+t0 : 1+t0+n] = xt[cob][:, 1:1+n]
                    epi(ps[:, :n], ot[:, :n], "gelu", bbb[:, cob:cob + 1],
                        res_ap=xts[cob][:, 1:1 + n])
                    nc.sync.dma_start(out=dst[cob, :, dst_off + t0:dst_off + t0 + n],
                                      in_=ot[:, :n])

        # ---------------- encoder ----------------
        w7 = wp.tile([7, 256], BF16, tag="w7")
        nc.sync.dma_start(out=w7, in_=P["w_enc_in"][:, :])
        b7 = load_b("b_enc_in", 2)
        for t0 in range(0, 27990, NT):
            n = min(NT, 27990 - t0)
            xt = ip.tile([7, NT], BF16, tag="x7")
            nc.sync.dma_start(
                out=xt[:, :n],
                in_=bass.AP(tensor=x.ap().tensor, offset=x.ap().offset + t0,
                            ap=[[1, 7], [1, n]]))
            for cob in range(2):
                ps = pp.tile([128, n], F32, tag="ps")
                nc.tensor.matmul(ps, w7[:, cob * 128:cob * 128 + 128],
                                 xt[:, :n], start=True, stop=True)
                ot = op.tile([128, n], BF16, tag="o")
                nc.scalar.activation(out=ot[:, :n], in_=ps, func=AF.Gelu,
                                     bias=b7[:, cob:cob + 1], scale=1.0)
                nc.sync.dma_start(out=S["h0"][cob, :, t0:t0 + n], in_=ot[:, :n])

        prev = S["h0"]
        for t, K, s, li, lo in ENC:
            if t.startswith("d"):
                conv(prev, S[t], f"w_{t}", f"b_{t}", K, s, lo, "none")
            else:
                resblock(prev, S[t], t, lo)
            prev = S[t]

        # enc_out 256->128 k1 -> z in SBUF
        wz = load_w("w_enc_out", 2 * 128)
        bz = load_b("b_enc_out", 1)
        xts = []
        for cc in range(2):
            xt = ip.tile([128, NFZ], BF16, tag=f"x{cc}")
            nc.sync.dma_start(out=xt[:, :NFZ], in_=S["r3"][cc, :, :NFZ])
            xts.append(xt)
        psz = pp.tile([128, NFZ], F32, tag="ps")
        for cc in range(2):
            nc.tensor.matmul(psz, wz[:, cc * 128:cc * 128 + 128],
                             xts[cc][:, :NFZ],
                             start=(cc == 0), stop=(cc == 1))
        z_sb = vp.tile([128, NFZ], F32, tag="z")
        nc.vector.tensor_scalar_add(z_sb, psz, bz[:, 0:1])

        # ---------------- VQ ----------------
        # zsq = ones.T @ (z*z)
        zz = vp.tile([128, NFZ], F32, tag="zz")
        nc.vector.tensor_mul(zz, z_sb, z_sb)
        ones = vp.tile([128, 1], F32, tag="ones")
        nc.vector.memset(ones, 1.0)
        ps1 = pt.tile([1, NFZ], F32, tag="pst")
        nc.tensor.matmul(ps1, ones, zz, start=True, stop=True)
        zsq_sb = vp.tile([1, NFZ], F32, tag="zsq")
        nc.vector.tensor_copy(zsq_sb, ps1)
        nc.sync.dma_start(out=zsq_o[:, :], in_=zsq_sb)

        ones84 = vp.tile([1, NFZ], F32, tag="o84")
        nc.vector.memset(ones84, 1.0)
        nd = vp.tile([NFZ, 4096], F32, tag="nd")
        for nk in range(8):
            cbt = wp.tile([128, NT], F32, tag="w")
            nc.sync.dma_start(out=cbt, in_=P["cbT2"][:, nk * NT:(nk + 1) * NT])
            cbsq = bp.tile([1, NT], F32, tag="cbsq")
            nc.sync.dma_start(out=cbsq, in_=P["cbnegsq"][:, nk * NT:(nk + 1) * NT])
            psd = pp.tile([NFZ, NT], F32, tag="ps")
            nc.tensor.matmul(psd, z_sb, cbt, start=True, stop=False)
            nc.tensor.matmul(psd, ones84, cbsq, start=False, stop=True)
            nc.vector.tensor_copy(nd[:, nk * NT:(nk + 1) * NT], psd)
        m8 = vp.tile([NFZ, 8], F32, tag="m8")
        nc.vector.max(m8, nd)
        i8 = vp.tile([NFZ, 8], U32, tag="i8")
        nc.vector.max_index(i8, m8, nd)
        nc.sync.dma_start(out=codes_o[:, :], in_=i8[:, 0:1])
        nc.sync.dma_start(out=vmax_o[:, :], in_=m8[:, 0:1])
        # one-hot in place
        nc.vector.tensor_scalar(out=nd, in0=nd, scalar1=m8[:, 0:1], scalar2=None,
                                op0=ALU.is_equal)
        eye = vp.tile([128, 128], F32, tag="eye")
        nc.sync.dma_start(out=eye, in_=P["eye"][:, :])
        psq = px.tile([128, NFZ], F32, tag="psq")
        for kt in range(32):
            pst = pt.tile([128, NFZ], F32, tag="pst")
            nc.tensor.transpose(pst, nd[:, kt * 128:(kt + 1) * 128], eye[:NFZ, :NFZ])
            oht = op.tile([128, NFZ], F32, tag="oht")
            nc.vector.tensor_copy(oht, pst)
            cbr = wp.tile([128, 128], F32, tag="w")
            nc.sync.dma_start(out=cbr, in_=P["cb"][kt * 128:(kt + 1) * 128, :])
            nc.tensor.matmul(psq, cbr, oht, start=(kt == 0), stop=(kt == 31))
        q_sb = vp.tile([128, NFZ], F32, tag="q")
        nc.vector.tensor_copy(q_sb, psq)

        # ---------------- decoder ----------------
        # dec_in k1 128->256, gelu -> A (pad1)
        wdi = load_w("w_dec_in", 2 * 128, dt=F32)
        bdi = load_b("b_dec_in", 2)
        for cob in range(2):
            ps = pp.tile([128, NFZ], F32, tag="ps")
            nc.tensor.matmul(ps, wdi[:, cob * 128:cob * 128 + 128],
                             q_sb, start=True, stop=True)
            ot = op.tile([128, NFZ], BF16, tag="o")
            nc.scalar.activation(out=ot[:, :NFZ], in_=ps, func=AF.Gelu,
                                 bias=bdi[:, cob:cob + 1], scale=1.0)
            nc.sync.dma_start(out=S["A"][cob, :, 1:1 + NFZ], in_=ot[:, :NFZ])

        prev = S["A"]
        for i in range(4):
            L = DEC_L[i]
            s = DEC_S[i]
            Lt = L + 1
            keep = UP_KEEP[i]
            B = S[f"B{i}"]
            C = S[f"C{i}"]
            # resblock same-pad: src padded (interior at off 1), dst B (interior off 1)
            resblock(prev, B, f"u{i}", L, src_off=0, dst_off=1, dt=BF16)
            # up conv, polyphase; B padded: interior [1, 1+L), reads t+e+m for
            # e in {-1,0}, m in {0,1}, t in [0, Lt) -> idx range [-1, L+2)
            padl = 1 if i < 3 else 3
            wu = load_w(f"w_up{i}", 2 * 2 * s * 2 * 128, dt=BF16)
            bu = load_b(f"b_up{i}", 2)
            p = 2 * s - 1 - s // 2
            ntp = min(NT, Lt)
            for t0 in range(0, Lt, ntp):
                n = min(ntp, Lt - t0)
                nc2 = n + (n % 2)
                span = nc2 + 2
                xts = []
                for cc in range(2):
                    xt = ip.tile([128, span], BF16, tag=f"x{cc}")
                    nc.sync.dma_start(out=xt[:, :span], in_=B[cc, :, t0:t0 + span])
                    xts.append(xt)
                ots = []
                for _cob in range(2):
                    ott = op.tile([128, s * ntp], BF16, tag="o")
                    ots.append(ott)
                nw = 0
                for r in range(s):
                    k0 = (p - r) % s
                    e0 = (r - p + k0) // s
                    # valid positions: t0 <= t < t0+n with s*t + r < keep
                    nv = min(n, -(-(keep - r) // s) - t0)
                    if nv <= 0:
                        continue
                    nw = max(nw, (nv - 1) * s + r + 1)
                    for cob in range(2):
                        ps = pp.tile([128, nc2], F32, tag="ps")
                        i_mm = 0
                        for cc in range(2):
                            for m in (0, 1):
                                k = k0 + s * m
                                nc.tensor.matmul(
                                    ps,
                                    wu[:, ((cc * 2 * s + k) * 2 + cob) * 128:
                                       ((cc * 2 * s + k) * 2 + cob) * 128 + 128],
                                    xts[cc][:, 1 + e0 + m:1 + e0 + m + nc2],
                                    start=(i_mm == 0), stop=(i_mm == 3))
                                i_mm += 1
                        nc.scalar.activation(
                            out=ots[cob][:, r:r + (nv - 1) * s + 1:s], in_=ps[:, :nv],
                            func=AF.Gelu, bias=bu[:, cob:cob + 1], scale=1.0)
                for cob in range(2):
                    nc.sync.dma_start(
                        out=C[cob, :, padl + s * t0:padl + s * t0 + nw],
                        in_=ots[cob][:, :nw])
            prev = C

        # dec_out: M=16 im2col, J=22 taps, tanh
        wd = wp.tile([128, 704], BF16, tag="w")
        nc.sync.dma_start(out=wd, in_=P["w_dec_out"][:, :])
        bd = bp.tile([16, 1], F32, tag="bd")
        nc.sync.dma_start(out=bd, in_=P["b_dec_out"][:, :])
        NA = 256
        C3 = S["C3"]
        for a0 in range(0, 1681, NA):
            na = min(NA, 1681 - a0)
            nac = na + (na % 2)
            span = 16 * (nac - 1) + 22
            xts = []
            for cc in range(2):
                xt = ip.tile([128, 16 * NA + 22], BF16, tag=f"x{cc}")
                nc.sync.dma_start(out=xt[:, :span],
                                  in_=C3[cc, :, 16 * a0:16 * a0 + span])
                xts.append(xt)
            ps = pp.tile([16, nac], F32, tag="ps")
            i_mm = 0
            for cc in range(2):
                for j in range(22):
                    nc.tensor.matmul(
                        ps,
                        wd[:, (cc * 22 + j) * 16:(cc * 22 + j) * 16 + 16],
                        xts[cc][:, j:j + 16 * (nac - 1) + 1:16],
                        start=(i_mm == 0), stop=(i_mm == 43))
                    i_mm += 1
            rt = op.tile([16, na], F32, tag="rt")
            nc.scalar.activation(out=rt[:, :na], in_=ps[:, :na], func=AF.Tanh,
                                 bias=bd[:, 0:1], scale=1.0)
            rv = recon_o.ap().rearrange("o (a b) -> o b a", b=16)
            nc.sync.dma_start(out=rv[0, :, a0:a0 + na], in_=rt[:, :na])

    nc.finalize()
    return nc


def _prep_inputs(waveform, params):
    p = {k: np.ascontiguousarray(np.asarray(v), np.float32) for k, v in params.items()}
    shared = {
        "w_enc_in": np.ascontiguousarray(p["enc_in_w"][:, 0, :].T).astype(ml_dtypes.bfloat16),
        "b_enc_in": _bprep(p["enc_in_b"]),
        "w_enc_out": _wprep(p["enc_out_w"]).astype(ml_dtypes.bfloat16),
        "b_enc_out": _bprep(p["enc_out_b"]),
        "w_dec_in": _wprep(p["dec_in_w"]),
        "b_dec_in": _bprep(p["dec_in_b"]),
        "w_dec_out": _dec_out_wprep(p["dec_out_w"]).astype(ml_dtypes.bfloat16),
        "b_dec_out": np.full((16, 1), p["dec_out_b"][0], np.float32),
        "cbT2": np.ascontiguousarray(2.0 * p["codebook"].T),
        "cbnegsq": np.ascontiguousarray(-np.sum(p["codebook"] ** 2, -1)[None, :]),
        "cb": p["codebook"],
        "eye": np.eye(128, dtype=np.float32),
    }
    for i in range(4):
        shared[f"w_d{i}"] = _wprep(p[f"enc{i}_down_w"]).astype(ml_dtypes.bfloat16)
        shared[f"b_d{i}"] = _bprep(p[f"enc{i}_down_b"])
        shared[f"w_r{i}a"] = _wprep(p[f"enc{i}_res1_w"]).astype(ml_dtypes.bfloat16)
        shared[f"b_r{i}a"] = _bprep(p[f"enc{i}_res1_b"])
        shared[f"w_r{i}b"] = _wprep(p[f"enc{i}_res2_w"]).astype(ml_dtypes.bfloat16)
        shared[f"b_r{i}b"] = _bprep(p[f"enc{i}_res2_b"])
        shared[f"w_u{i}a"] = _wprep(p[f"dec{i}_res1_w"]).astype(ml_dtypes.bfloat16)
        shared[f"b_u{i}a"] = _bprep(p[f"dec{i}_res1_b"])
        shared[f"w_u{i}b"] = _wprep(p[f"dec{i}_res2_w"]).astype(ml_dtypes.bfloat16)
        shared[f"b_u{i}b"] = _bprep(p[f"dec{i}_res2_b"])
        shared[f"w_up{i}"] = _wprep(p[f"dec{i}_up_w"]).astype(ml_dtypes.bfloat16)
        shared[f"b_up{i}"] = _bprep(p[f"dec{i}_up_b"])

    wf = np.asarray(waveform, np.float32)
    in_maps = []
    for core in range(8):
        b, half = core // 2, core % 2
        g0 = 320 * F0S[half] - 743
        xw = np.zeros((1, LIN), np.float32)
        lo, hi = max(0, g0), min(48000, g0 + LIN)
        xw[0, lo - g0:hi - g0] = wf[b, lo:hi]
        m = dict(shared)
        m["x"] = xw.astype(ml_dtypes.bfloat16)
        in_maps.append(m)
    return in_maps


def kernel(waveform, params, _trace=False):
    if _G.nc is None:
        _G.nc = _build()
    in_maps = _prep_inputs(waveform, params)
    res = run_bass_kernel_spmd(_G.nc, in_maps, core_ids=list(range(8)),
                               trace=_trace)
    kernel._last = res
    wf = np.asarray(waveform, np.float32)
    recon = np.zeros((4, T_FULL), np.float32)
    codes = np.zeros((4, F_FULL), np.int32)
    dsum = 0.0
    for core in range(8):
        b, half = core // 2, core % 2
        r = res.results[core]
        rw = r["recon"][0]
        cw = r["codes_u"][:, 0].astype(np.int32)
        dm = r["zsq"][0] - r["vmax"][:, 0]
        ks, ke = KEEP_FRAMES[half]
        codes[b, F0S[half] + ks:F0S[half] + ke] = cw[ks:ke]
        dsum += float(np.sum(dm[ks:ke]))
        rs, re = KEEP_RECON[half]
        recon[b, RECON_OFF[half] + rs:RECON_OFF[half] + re] = rw[rs:re]
    vq_loss = np.float32(1.25 * dsum / (4 * F_FULL * 128))
    recon_loss = np.float32(np.mean(np.abs(recon[:, :48000] - wf[:, :48000])))
    return recon, codes, recon_loss, vq_loss


# revision 22
# speedup vs baseline: 1.0004x; 1.0004x over previous
"""Trainium2 Bass kernel for nn_AudioCodec (conv encoder + VQ + conv decoder).

Sharding: 8 cores = 4 batch x 2 time-halves. Each core gets a 27996-sample
zero-padded window of its sample, runs the full encoder (valid convs) to 84
z-frames, VQ against the 4096x128 codebook, decodes (same-padded convs,
polyphase transposed convs) to a 26888-sample recon window, and the host
stitches exact interior regions. Zero cross-core communication.

All convs run as TensorE matmuls in float32r (full PE rate); VQ distance /
one-hot matmuls in plain f32 so argmin matches the reference bit-for-bit.
"""
import sys

sys.path.insert(0, "/opt/trn_rl_repo")

import numpy as np
import ml_dtypes

import concourse.bass as bass
import concourse.mybir as mybir
from concourse import bacc
from concourse.tile import TileContext
from concourse.bass_utils import run_bass_kernel_spmd

F32 = mybir.dt.float32
F32R = mybir.dt.float32r
U32 = mybir.dt.uint32
BF16 = mybir.dt.bfloat16
AF = mybir.ActivationFunctionType
ALU = mybir.AluOpType

STRIDES = (2, 4, 5, 8)
LIN = 27996
NFZ = 84
F0S = (0, 67)
KEEP_FRAMES = ((0, 76), (9, 84))
KEEP_RECON = ((0, 24320), (2880, 26888))
RECON_OFF = (0, 21440)
T_FULL = 48328
F_FULL = 151
NT = 512

# encoder chain lengths (valid convs)
ENC = [  # (tag, K, stride, Lin, Lout)
    ("d0", 4, 2, 27990, 13994),
    ("r0", 3, 1, 13994, 13992),
    ("d1", 8, 4, 13992, 3497),
    ("r1", 3, 1, 3497, 3495),
    ("d2", 10, 5, 3495, 698),
    ("r2", 3, 1, 698, 696),
    ("d3", 16, 8, 696, 86),
    ("r3", 3, 1, 86, 84),
]
# decoder: resblock length then up-keep length per stage
DEC_L = [84, 672, 3361, 13444]
DEC_S = [8, 5, 4, 2]
UP_KEEP = [672, 3361, 13444, 26888]


def _wprep(w):
    """[O,I,K] -> [128, (I/128)*K*O] matmul-ready lhsT blocks."""
    O, I, K = w.shape
    cci = I // 128
    wp = w.transpose(1, 2, 0).reshape(cci, 128, K * O)
    return np.ascontiguousarray(wp.transpose(1, 0, 2).reshape(128, cci * K * O))


def _bprep(b):
    """[C] -> [128, C/128]"""
    return np.ascontiguousarray(b.reshape(-1, 128).T)


def _dec_out_wprep(w):
    """dec_out w [1,256,7] -> M=16 im2col weights [128, 2*22*16]."""
    M, J = 16, 22
    wd = np.zeros((2, 128, J, M), np.float32)
    for cc in range(2):
        for j in range(J):
            for b in range(M):
                k = j - b
                if 0 <= k < 7:
                    wd[cc, :, j, b] = w[0, cc * 128:(cc + 1) * 128, k]
    return np.ascontiguousarray(
        wd.transpose(1, 0, 2, 3).reshape(128, 2 * J * M))


class _G:
    nc = None
    names = None


def _build():
    nc = bacc.Bacc("TRN2", target_bir_lowering=False)
    P = {}

    def inp(name, shape, dtype=F32):
        P[name] = nc.declare_dram_parameter(name, list(shape), dtype, isOutput=False)
        return P[name]

    x = inp("x", [1, LIN], BF16)
    inp("w_enc_in", [7, 256], BF16)
    inp("b_enc_in", [128, 2])
    for t, K, s, li, lo in ENC:
        if t.startswith("d"):
            inp(f"w_{t}", [128, 2 * K * 256], BF16)
            inp(f"b_{t}", [128, 2])
        else:
            inp(f"w_{t}a", [128, 2 * 3 * 256], BF16)
            inp(f"b_{t}a", [128, 2])
            inp(f"w_{t}b", [128, 2 * 1 * 256], BF16)
            inp(f"b_{t}b", [128, 2])
    inp("w_enc_out", [128, 2 * 128], BF16)
    inp("b_enc_out", [128, 1])
    inp("w_dec_in", [128, 256])
    inp("b_dec_in", [128, 2])
    for i in range(4):
        inp(f"w_u{i}a", [128, 2 * 3 * 256], BF16)
        inp(f"b_u{i}a", [128, 2])
        inp(f"w_u{i}b", [128, 2 * 1 * 256], BF16)
        inp(f"b_u{i}b", [128, 2])
        K = 2 * DEC_S[i]
        inp(f"w_up{i}", [128, 2 * K * 256], BF16)
        inp(f"b_up{i}", [128, 2])
    inp("w_dec_out", [128, 2 * 22 * 16], BF16)
    inp("b_dec_out", [16, 1])
    inp("cbT2", [128, 4096])
    inp("cbnegsq", [1, 4096])
    inp("cb", [4096, 128])
    inp("eye", [128, 128])

    recon_o = nc.declare_dram_parameter("recon", [1, 26896], F32, isOutput=True)
    codes_o = nc.declare_dram_parameter("codes_u", [84, 1], U32, isOutput=True)
    vmax_o = nc.declare_dram_parameter("vmax", [84, 1], F32, isOutput=True)
    zsq_o = nc.declare_dram_parameter("zsq", [1, 84], F32, isOutput=True)

    # DRAM staging (per-core local scratch)
    S = {}
    S["h0"] = nc.dram_tensor("s_h0", [2, 128, 27990 + 16], BF16)
    for t, K, s, li, lo in ENC:
        S[t] = nc.dram_tensor("s_" + t, [2, 128, lo + 16], BF16)
    # decoder padded stages
    S["A"] = nc.dram_tensor("s_A", [2, 128, 84 + 2 + 16], BF16)          # dec_in out, pad1
    for i in range(4):
        L = DEC_L[i]
        S[f"B{i}"] = nc.dram_tensor(f"s_B{i}", [2, 128, L + 3 + 16], BF16)  # res out: padl1,padr2
        s = DEC_S[i]
        Lt = L + 1
        if i < 3:
            S[f"C{i}"] = nc.dram_tensor(f"s_C{i}", [2, 128, 1 + s * Lt + 1 + 16], BF16)
        else:
            S[f"C{i}"] = nc.dram_tensor(f"s_C{i}", [2, 128, 3 + s * Lt + 11 + 16], BF16)

    from contextlib import ExitStack
    with TileContext(nc) as tc, ExitStack() as stk:
        wp = stk.enter_context(tc.tile_pool(name="wp", bufs=3))
        bp = stk.enter_context(tc.tile_pool(name="bp", bufs=4))
        ip = stk.enter_context(tc.tile_pool(name="ip", bufs=5))
        op = stk.enter_context(tc.tile_pool(name="op", bufs=4))
        pp = stk.enter_context(tc.tile_pool(name="pp", bufs=4, space="PSUM"))
        pq = stk.enter_context(tc.tile_pool(name="pq", bufs=2, space="PSUM"))
        pt = stk.enter_context(tc.tile_pool(name="pt", bufs=1, space="PSUM"))
        px = stk.enter_context(tc.tile_pool(name="px", bufs=1, space="PSUM"))
        vp = stk.enter_context(tc.tile_pool(name="vp", bufs=1))

        zt = vp.tile([128, 16], F32, tag="zt")
        nc.vector.memset(zt, 0.0)
        ztb = vp.tile([128, 16], BF16, tag="ztb")
        nc.vector.memset(ztb, 0.0)

        def zero_cols(stage, cc, c0, c1):
            if c1 > c0:
                if stage.dtype == BF16:
                    nc.sync.dma_start(out=stage[cc, :, c0:c1], in_=ztb[:, : c1 - c0])
                else:
                    nc.sync.dma_start(out=stage[cc, :, c0:c1], in_=zt[:, : c1 - c0].bitcast(F32R))

        for _sname, _st in S.items():
            _sz = _st.shape[2]
            for cc in range(2):
                zero_cols(_st, cc, _sz - 16, _sz)
        # decoder pads: A [0,1)+[85,86); B_i [0,1)+[1+L,3+L); C_i [0,padl) and
        # [padl+keep, end-16) (up convs only ever write [padl, padl+keep))
        for cc in range(2):
            zero_cols(S["A"], cc, 0, 1)
            zero_cols(S["A"], cc, 1 + NFZ, 2 + NFZ)
            for i in range(4):
                L = DEC_L[i]
                zero_cols(S[f"B{i}"], cc, 0, 1)
                zero_cols(S[f"B{i}"], cc, 1 + L, 3 + L)
                padl = 1 if i < 3 else 3
                Csz = S[f"C{i}"].shape[2]
                zero_cols(S[f"C{i}"], cc, 0, padl)
                zero_cols(S[f"C{i}"], cc, padl + UP_KEEP[i], Csz - 16)

        def load_w(name, cols, dt=BF16):
            w = wp.tile([128, cols], dt, tag="w")
            nc.sync.dma_start(out=w, in_=P[name][:, :cols])
            return w

        def load_b(name, cco):
            b = bp.tile([128, cco], F32, tag="b")
            nc.sync.dma_start(out=b, in_=P[name][:, :cco])
            return b

        def epi(ps, dst_ap, act, bias_ap, res_ap=None):
            if res_ap is not None:
                tmp = op.tile(list(ps.shape), F32, tag="tmp")
                nc.vector.tensor_add(tmp, ps, res_ap.bitcast(F32) if res_ap.dtype == F32R else res_ap)
                nc.scalar.activation(out=dst_ap, in_=tmp, func=AF.Gelu,
                                     bias=bias_ap, scale=1.0)
            elif act == "gelu":
                nc.scalar.activation(out=dst_ap, in_=ps, func=AF.Gelu,
                                     bias=bias_ap, scale=1.0)
            elif act == "tanh":
                nc.scalar.activation(out=dst_ap, in_=ps, func=AF.Tanh,
                                     bias=bias_ap, scale=1.0)
            else:
                nc.vector.tensor_scalar_add(dst_ap, ps, bias_ap)

        def conv(src, dst, wname, bname, K, stride, Lout, act,
                 cci=2, cco=2, src_off=0, dst_off=0):
            """standard conv: src/dst dram [cc,128,*]; valid over src."""
            w = load_w(wname, cci * K * cco * 128)
            bb = load_b(bname, cco)
            for t0 in range(0, Lout, NT):
                n = min(NT, Lout - t0)
                nc2 = n + (n % 2)
                span = (nc2 - 1) * stride + K
                xts = []
                for cc in range(cci):
                    xt = ip.tile([128, span], BF16, tag=f"x{cc}")
                    nc.sync.dma_start(
                        out=xt[:, :span],
                        in_=src[cc, :, src_off + t0 * stride:
                                src_off + t0 * stride + span])
                    xts.append(xt)
                for cob in range(cco):
                    ps = pp.tile([128, nc2], F32, tag="ps")
                    nmm = cci * K
                    i = 0
                    for cc in range(cci):
                        for k in range(K):
                            nc.tensor.matmul(
                                ps,
                                w[:, ((cc * K + k) * cco + cob) * 128:
                                  ((cc * K + k) * cco + cob) * 128 + 128],
                                xts[cc][:, k:k + (nc2 - 1) * stride + 1:stride],
                                start=(i == 0), stop=(i == nmm - 1))
                            i += 1
                    ot = op.tile([128, nc2], BF16, tag="o")
                    epi(ps[:, :n], ot[:, :n], act, bb[:, cob:cob + 1])
                    nc.sync.dma_start(out=dst[cob, :, dst_off + t0:dst_off + t0 + n],
                                      in_=ot[:, :n])

        def resblock(src, dst, tag, Lout, src_off=0, dst_off=0, same=False, dt=BF16):
            """conv3(+gelu) -> conv1 -> +res -> gelu. same=False: valid (src Lin=Lout+2)."""
            wa = load_w(f"w_{tag}a", 2 * 3 * 2 * 128, dt=dt)
            ba = load_b(f"b_{tag}a", 2)
            wb = load_w(f"w_{tag}b", 2 * 1 * 2 * 128, dt=dt)
            bbb = load_b(f"b_{tag}b", 2)
            for t0 in range(0, Lout, NT):
                n = min(NT, Lout - t0)
                nc2 = n + (n % 2)
                span = nc2 + 2
                xts = []
                for cc in range(2):
                    xt = ip.tile([128, span], dt, tag=f"x{cc}")
                    nc.sync.dma_start(out=xt[:, :span],
                                      in_=src[cc, :, src_off + t0:src_off + t0 + span])
                    xts.append(xt)
                ats = []
                for cob in range(2):
                    ps = pp.tile([128, nc2], F32, tag="ps")
                    i = 0
                    for cc in range(2):
                        for k in range(3):
                            nc.tensor.matmul(
                                ps,
                                wa[:, ((cc * 3 + k) * 2 + cob) * 128:
                                   ((cc * 3 + k) * 2 + cob) * 128 + 128],
                                xts[cc][:, k:k + nc2],
                                start=(i == 0), stop=(i == 5))
                            i += 1
                    at = op.tile([128, nc2], dt, tag="a")
                    nc.scalar.activation(out=at[:, :nc2], in_=ps, func=AF.Gelu,
                                         bias=ba[:, cob:cob + 1], scale=1.0)
                    ats.append(at)
                for cob in range(2):
                    ps = pq.tile([128, nc2], F32, tag="ps2")
                    for cc in range(2):
                        nc.tensor.matmul(
                            ps,
                            wb[:, (cc * 2 + cob) * 128:(cc * 2 + cob) * 128 + 128],
                            ats[cc][:, :nc2],
                            start=(cc == 0), stop=(cc == 1))
                    ot = op.tile([128, nc2], dt, tag="o")
                    # residual = src[cob][:, 1+t0 : 1+t0+n] = xt[cob][:, 1:1+n]
                    epi(ps[:, :n], ot[:, :n], "gelu", bbb[:, cob:cob + 1],
                        res_ap=xts[cob][:, 1:1 + n])
                    nc.sync.dma_start(out=dst[cob, :, dst_off + t0:dst_off + t0 + n],
                                      in_=ot[:, :n])

        # ---------------- encoder ----------------
        w7 = wp.tile([7, 256], BF16, tag="w7")
        nc.sync.dma_start(out=w7, in_=P["w_enc_in"][:, :])
        b7 = load_b("b_enc_in", 2)
        for t0 in range(0, 27990, NT):
            n = min(NT, 27990 - t0)
            xt = ip.tile([7, NT], BF16, tag="x7")
            nc.sync.dma_start(
                out=xt[:, :n],
                in_=bass.AP(tensor=x.ap().tensor, offset=x.ap().offset + t0,
                            ap=[[1, 7], [1, n]]))
            for cob in range(2):
                ps = pp.tile([128, n], F32, tag="ps")
                nc.tensor.matmul(ps, w7[:, cob * 128:cob * 128 + 128],
                                 xt[:, :n], start=True, stop=True)
                ot = op.tile([128, n], BF16, tag="o")
                nc.scalar.activation(out=ot[:, :n], in_=ps, func=AF.Gelu,
                                     bias=b7[:, cob:cob + 1], scale=1.0)
                nc.sync.dma_start(out=S["h0"][cob, :, t0:t0 + n], in_=ot[:, :n])

        prev = S["h0"]
        for t, K, s, li, lo in ENC:
            if t.startswith("d"):
                conv(prev, S[t], f"w_{t}", f"b_{t}", K, s, lo, "none")
            else:
                resblock(prev, S[t], t, lo)
            prev = S[t]

        # enc_out 256->128 k1 -> z in SBUF
        wz = load_w("w_enc_out", 2 * 128)
        bz = load_b("b_enc_out", 1)
        xts = []
        for cc in range(2):
            xt = ip.tile([128, NFZ], BF16, tag=f"x{cc}")
            nc.sync.dma_start(out=xt[:, :NFZ], in_=S["r3"][cc, :, :NFZ])
            xts.append(xt)
        psz = pp.tile([128, NFZ], F32, tag="ps")
        for cc in range(2):
            nc.tensor.matmul(psz, wz[:, cc * 128:cc * 128 + 128],
                             xts[cc][:, :NFZ],
                             start=(cc == 0), stop=(cc == 1))
        z_sb = vp.tile([128, NFZ], F32, tag="z")
        nc.vector.tensor_scalar_add(z_sb, psz, bz[:, 0:1])

        # ---------------- VQ ----------------
        # zsq = ones.T @ (z*z)
        zz = vp.tile([128, NFZ], F32, tag="zz")
        nc.vector.tensor_mul(zz, z_sb, z_sb)
        ones = vp.tile([128, 1], F32, tag="ones")
        nc.vector.memset(ones, 1.0)
        ps1 = pt.tile([1, NFZ], F32, tag="pst")
        nc.tensor.matmul(ps1, ones, zz, start=True, stop=True)
        zsq_sb = vp.tile([1, NFZ], F32, tag="zsq")
        nc.vector.tensor_copy(zsq_sb, ps1)
        nc.sync.dma_start(out=zsq_o[:, :], in_=zsq_sb)

        ones84 = vp.tile([1, NFZ], F32, tag="o84")
        nc.vector.memset(ones84, 1.0)
        nd = vp.tile([NFZ, 4096], F32, tag="nd")
        for nk in range(8):
            cbt = wp.tile([128, NT], F32, tag="w")
            nc.sync.dma_start(out=cbt, in_=P["cbT2"][:, nk * NT:(nk + 1) * NT])
            cbsq = bp.tile([1, NT], F32, tag="cbsq")
            nc.sync.dma_start(out=cbsq, in_=P["cbnegsq"][:, nk * NT:(nk + 1) * NT])
            psd = pp.tile([NFZ, NT], F32, tag="ps")
            nc.tensor.matmul(psd, z_sb, cbt, start=True, stop=False)
            nc.tensor.matmul(psd, ones84, cbsq, start=False, stop=True)
            nc.vector.tensor_copy(nd[:, nk * NT:(nk + 1) * NT], psd)
        m8 = vp.tile([NFZ, 8], F32, tag="m8")
        nc.vector.max(m8, nd)
        i8 = vp.tile([NFZ, 8], U32, tag="i8")
        nc.vector.max_index(i8, m8, nd)
        nc.sync.dma_start(out=codes_o[:, :], in_=i8[:, 0:1])
        nc.sync.dma_start(out=vmax_o[:, :], in_=m8[:, 0:1])
        # one-hot in place
        nc.vector.tensor_scalar(out=nd, in0=nd, scalar1=m8[:, 0:1], scalar2=None,
                                op0=ALU.is_equal)
        eye = vp.tile([128, 128], F32, tag="eye")
        nc.sync.dma_start(out=eye, in_=P["eye"][:, :])
        psq = px.tile([128, NFZ], F32, tag="psq")
        for kt in range(32):
            pst = pt.tile([128, NFZ], F32, tag="pst")
            nc.tensor.transpose(pst, nd[:, kt * 128:(kt + 1) * 128], eye[:NFZ, :NFZ])
            oht = op.tile([128, NFZ], F32, tag="oht")
            nc.vector.tensor_copy(oht, pst)
            cbr = wp.tile([128, 128], F32, tag="w")
            nc.sync.dma_start(out=cbr, in_=P["cb"][kt * 128:(kt + 1) * 128, :])
            nc.tensor.matmul(psq, cbr, oht, start=(kt == 0), stop=(kt == 31))
        q_sb = vp.tile([128, NFZ], F32, tag="q")
        nc.vector.tensor_copy(q_sb, psq)

        # ---------------- decoder ----------------
        # dec_in k1 128->256, gelu -> A (pad1)
        wdi = load_w("w_dec_in", 2 * 128, dt=F32)
        bdi = load_b("b_dec_in", 2)
        for cob in range(2):
            ps = pp.tile([128, NFZ], F32, tag="ps")
            nc.tensor.matmul(ps, wdi[:, cob * 128:cob * 128 + 128],
                             q_sb, start=True, stop=True)
            ot = op.tile([128, NFZ], BF16, tag="o")
            nc.scalar.activation(out=ot[:, :NFZ], in_=ps, func=AF.Gelu,
                                 bias=bdi[:, cob:cob + 1], scale=1.0)
            nc.sync.dma_start(out=S["A"][cob, :, 1:1 + NFZ], in_=ot[:, :NFZ])

        prev = S["A"]
        for i in range(4):
            L = DEC_L[i]
            s = DEC_S[i]
            Lt = L + 1
            keep = UP_KEEP[i]
            B = S[f"B{i}"]
            C = S[f"C{i}"]
            # resblock same-pad: src padded (interior at off 1), dst B (interior off 1)
            resblock(prev, B, f"u{i}", L, src_off=0, dst_off=1, dt=BF16)
            # up conv, polyphase; B padded: interior [1, 1+L), reads t+e+m for
            # e in {-1,0}, m in {0,1}, t in [0, Lt) -> idx range [-1, L+2)
            padl = 1 if i < 3 else 3
            wu = load_w(f"w_up{i}", 2 * 2 * s * 2 * 128, dt=BF16)
            bu = load_b(f"b_up{i}", 2)
            p = 2 * s - 1 - s // 2
            ntp = min(NT, Lt)
            for t0 in range(0, Lt, ntp):
                n = min(ntp, Lt - t0)
                nc2 = n + (n % 2)
                span = nc2 + 2
                xts = []
                for cc in range(2):
                    xt = ip.tile([128, span], BF16, tag=f"x{cc}")
                    nc.sync.dma_start(out=xt[:, :span], in_=B[cc, :, t0:t0 + span])
                    xts.append(xt)
                ots = []
                for _cob in range(2):
                    ott = op.tile([128, s * ntp], BF16, tag="o")
                    ots.append(ott)
                nw = 0
                for r in range(s):
                    k0 = (p - r) % s
                    e0 = (r - p + k0) // s
                    # valid positions: t0 <= t < t0+n with s*t + r < keep
                    nv = min(n, -(-(keep - r) // s) - t0)
                    if nv <= 0:
                        continue
                    nw = max(nw, (nv - 1) * s + r + 1)
                    for cob in range(2):
                        ps = pp.tile([128, nc2], F32, tag="ps")
                        i_mm = 0
                        for cc in range(2):
                            for m in (0, 1):
                                k = k0 + s * m
                                nc.tensor.matmul(
                                    ps,
                                    wu[:, ((cc * 2 * s + k) * 2 + cob) * 128:
                                       ((cc * 2 * s + k) * 2 + cob) * 128 + 128],
                                    xts[cc][:, 1 + e0 + m:1 + e0 + m + nc2],
                                    start=(i_mm == 0), stop=(i_mm == 3))
                                i_mm += 1
                        nc.scalar.activation(
                            out=ots[cob][:, r:r + (nv - 1) * s + 1:s], in_=ps[:, :nv],
                            func=AF.Gelu, bias=bu[:, cob:cob + 1], scale=1.0)
                for cob in range(2):
                    nc.sync.dma_start(
                        out=C[cob, :, padl + s * t0:padl + s * t0 + nw],
                        in_=ots[cob][:, :nw])
            prev = C

        # dec_out: M=16 im2col, J=22 taps, tanh
        wd = wp.tile([128, 704], BF16, tag="w")
        nc.sync.dma_start(out=wd, in_=P["w_dec_out"][:, :])
        bd = bp.tile([16, 1], F32, tag="bd")
        nc.sync.dma_start(out=bd, in_=P["b_dec_out"][:, :])
        NA = 256
        C3 = S["C3"]
        for a0 in range(0, 1681, NA):
            na = min(NA, 1681 - a0)
            nac = na + (na % 2)
            span = 16 * (nac - 1) + 22
            xts = []
            for cc in range(2):
                xt = ip.tile([128, 16 * NA + 22], BF16, tag=f"x{cc}")
                nc.sync.dma_start(out=xt[:, :span],
                                  in_=C3[cc, :, 16 * a0:16 * a0 + span])
                xts.append(xt)
            ps = pp.tile([16, nac], F32, tag="ps")
            i_mm = 0
            for cc in range(2):
                for j in range(22):
                    nc.tensor.matmul(
                        ps,
                        wd[:, (cc * 22 + j) * 16:(cc * 22 + j) * 16 + 16],
                        xts[cc][:, j:j + 16 * (nac - 1) + 1:16],
                        start=(i_mm == 0), stop=(i_mm == 43))
                    i_mm += 1
            rt = op.tile([16, na], F32, tag="rt")
            nc.scalar.activation(out=rt[:, :na], in_=ps[:, :na], func=AF.Tanh,
                                 bias=bd[:, 0:1], scale=1.0)
            rv = recon_o.ap().rearrange("o (a b) -> o b a", b=16)
            nc.sync.dma_start(out=rv[0, :, a0:a0 + na], in_=rt[:, :na])

    nc.finalize()
    return nc


def _prep_inputs(waveform, params):
    p = {k: np.ascontiguousarray(np.asarray(v), np.float32) for k, v in params.items()}
    shared = {
        "w_enc_in": np.ascontiguousarray(p["enc_in_w"][:, 0, :].T).astype(ml_dtypes.bfloat16),
        "b_enc_in": _bprep(p["enc_in_b"]),
        "w_enc_out": _wprep(p["enc_out_w"]).astype(ml_dtypes.bfloat16),
        "b_enc_out": _bprep(p["enc_out_b"]),
        "w_dec_in": _wprep(p["dec_in_w"]),
        "b_dec_in": _bprep(p["dec_in_b"]),
        "w_dec_out": _dec_out_wprep(p["dec_out_w"]).astype(ml_dtypes.bfloat16),
        "b_dec_out": np.full((16, 1), p["dec_out_b"][0], np.float32),
        "cbT2": np.ascontiguousarray(2.0 * p["codebook"].T),
        "cbnegsq": np.ascontiguousarray(-np.sum(p["codebook"] ** 2, -1)[None, :]),
        "cb": p["codebook"],
        "eye": np.eye(128, dtype=np.float32),
    }
    for i in range(4):
        shared[f"w_d{i}"] = _wprep(p[f"enc{i}_down_w"]).astype(ml_dtypes.bfloat16)
        shared[f"b_d{i}"] = _bprep(p[f"enc{i}_down_b"])
        shared[f"w_r{i}a"] = _wprep(p[f"enc{i}_res1_w"]).astype(ml_dtypes.bfloat16)
        shared[f"b_r{i}a"] = _bprep(p[f"enc{i}_res1_b"])
        shared[f"w_r{i}b"] = _wprep(p[f"enc{i}_res2_w"]).astype(ml_dtypes.bfloat16)
        shared[f"b_r{i}b"] = _bprep(p[f"enc{i}_res2_b"])
        shared[f"w_u{i}a"] = _wprep(p[f"dec{i}_res1_w"]).astype(ml_dtypes.bfloat16)
        shared[f"b_u{i}a"] = _bprep(p[f"dec{i}_res1_b"])
        shared[f"w_u{i}b"] = _wprep(p[f"dec{i}_res2_w"]).astype(ml_dtypes.bfloat16)
        shared[f"b_u{i}b"] = _bprep(p[f"dec{i}_res2_b"])
        shared[f"w_up{i}"] = _wprep(p[f"dec{i}_up_w"]).astype(ml_dtypes.bfloat16)
        shared[f"b_up{i}"] = _bprep(p[f"dec{i}_up_b"])

    wf = np.asarray(waveform, np.float32)
    in_maps = []
    for core in range(8):
        b, half = core // 2, core % 2
        g0 = 320 * F0S[half] - 743
        xw = np.zeros((1, LIN), np.float32)
        lo, hi = max(0, g0), min(48000, g0 + LIN)
        xw[0, lo - g0:hi - g0] = wf[b, lo:hi]
        m = dict(shared)
        m["x"] = xw.astype(ml_dtypes.bfloat16)
        in_maps.append(m)
    return in_maps


def kernel(waveform, params, _trace=False):
    if _G.nc is None:
        _G.nc = _build()
    in_maps = _prep_inputs(waveform, params)
    res = run_bass_kernel_spmd(_G.nc, in_maps, core_ids=list(range(8)),
                               trace=_trace)
    kernel._last = res
    wf = np.asarray(waveform, np.float32)
    recon = np.zeros((4, T_FULL), np.float32)
    codes = np.zeros((4, F_FULL), np.int32)
    dsum = 0.0
    for core in range(8):
        b, half = core // 2, core % 2
        r = res.results[core]
        rw = r["recon"][0]
        cw = r["codes_u"][:, 0].astype(np.int32)
        dm = r["zsq"][0] - r["vmax"][:, 0]
        ks, ke = KEEP_FRAMES[half]
        codes[b, F0S[half] + ks:F0S[half] + ke] = cw[ks:ke]
        dsum += float(np.sum(dm[ks:ke]))
        rs, re = KEEP_RECON[half]
        recon[b, RECON_OFF[half] + rs:RECON_OFF[half] + re] = rw[rs:re]
    vq_loss = np.float32(1.25 * dsum / (4 * F_FULL * 128))
    recon_loss = np.float32(np.mean(np.abs(recon[:, :48000] - wf[:, :48000])))
    return recon, codes, recon_loss, vq_loss
